# revision 1
# baseline (speedup 1.0000x reference)
"""GAT 2-layer node classifier on 8 Trainium2 NeuronCores.

Strategy (1D node partitioning, DistDGL-style):
  - dst nodes sharded contiguously across 8 cores (12500 each)
  - host: sort each core's edges by dst, order dst nodes by in-degree,
    pack 128 nodes per "group" with in-edges padded (to the group's max
    degree) along the free dimension; padded slots point to a dummy
    table row whose attention logit is -1e30 so it contributes 0.
  - device per layer: gather [feat | el] rows by src via indirect DMA,
    segment softmax per node (node-per-partition layout), weighted
    feature sum on VectorE, layer-2 projection fused into layer-1 loop,
    AllGather of the per-shard gather table between layers.
"""

import os
import sys
import types

import numpy as np

# ---------------------------------------------------------------------------
# environment shims (self-contained: only touches in-process state)
# ---------------------------------------------------------------------------


def _ensure_axon_hooks():
    """concourse.bass_utils imports antenv.axon_hooks when tracing under
    axon; some images lack the module. Provide an in-process shim."""
    try:
        import antenv.axon_hooks  # noqa: F401
        return
    except Exception:
        pass
    try:
        import antenv
    except Exception:
        return
    mod = types.ModuleType("antenv.axon_hooks")
    mod._hook = None

    def set_axon_ntff_profile_hook(hook):
        mod._hook = hook

    def get_axon_ntff_profile_hook():
        return mod._hook

    mod.set_axon_ntff_profile_hook = set_axon_ntff_profile_hook
    mod.get_axon_ntff_profile_hook = get_axon_ntff_profile_hook
    sys.modules["antenv.axon_hooks"] = mod
    antenv.axon_hooks = mod


_ensure_axon_hooks()

import concourse.bass as bass          # noqa: E402
import concourse.mybir as mybir        # noqa: E402
import concourse.tile as tile          # noqa: E402
from concourse.vector_clock import ScopedClock  # noqa: E402
from concourse.bass_utils import run_bass_kernel_spmd  # noqa: E402

F32 = mybir.dt.float32
I32 = mybir.dt.int32
AF = mybir.ActivationFunctionType
OP = mybir.AluOpType
AX = mybir.AxisListType


def _patched_drain_and_barrier(self, tick_clock, wait_clock):
    # this walrus build rejects multi-wait instructions; emit one wait per
    # nop before the tail drain instead of stacking them on the drain.
    nc = self.nc
    probe = nc.sync.nop(nofuse=True)
    wait_clock.add_sem_waits(probe.ins, ScopedClock({None: tick_clock.global_clock}))
    waits = list(probe.ins.sync_info.on_wait or []) if probe.ins.sync_info else []
    if waits:
        probe.ins.sync_info = mybir.SyncInfo(on_wait=[waits[0]], on_update=[])
        for w in waits[1:]:
            nop = nc.sync.nop(nofuse=True)
            nop.ins.sync_info = mybir.SyncInfo(on_wait=[w], on_update=[])
    nc.sync.drain()
    nc.all_engine_barrier()
    popped = nc._tile_sem_poison_stack.pop()
    assert popped is self._sem_poison
    nc.clear_and_free_semaphores(list(self.sems.allocated().values()))
    nc.all_engine_barrier()


tile.TileContext._drain_and_barrier = _patched_drain_and_barrier


def _split_waits(nc, max_waits=1):
    """Post-pass: any instruction carrying more than max_waits sem-waits gets
    preceding same-engine NoOps carrying the excess."""
    uid = [0]
    for f in nc.m.functions:
        for bb in f.blocks:
            new_insts = []
            for inst in bb.instructions:
                si = getattr(inst, "sync_info", None)
                if si is not None and si.on_wait and len(si.on_wait) > max_waits:
                    waits = list(si.on_wait)
                    excess, keep = waits[:-max_waits], waits[-max_waits:]
                    for i in range(0, len(excess), max_waits):
                        uid[0] += 1
                        new_insts.append(mybir.InstNoOp(
                            name=f"waitsplit-{uid[0]}-{inst.name}",
                            sync_info=mybir.SyncInfo(
                                on_wait=excess[i:i + max_waits], on_update=[]),
                            bass_nofuse=True,
                            engine=inst.engine,
                        ))
                    inst.sync_info = mybir.SyncInfo(
                        on_wait=keep, on_update=list(si.on_update or []))
                new_insts.append(inst)
            bb.instructions = new_insts


# ---------------------------------------------------------------------------
# problem constants (hardcoded per spec)
# ---------------------------------------------------------------------------
N_CORES = 8
V = 100000            # nodes
S = V // N_CORES      # nodes per core shard
F_IN = 256
H1, D1 = 8, 8         # layer-1 heads x dim
HD1 = H1 * D1         # 64
C2 = 40               # classes (layer-2 single head)
ROW1 = HD1 + H1       # 72: [feat64 | el8]
ROW2 = C2 + 2         # 42: [feat40 | el1 | pad]
NEG_SLOPE = 0.2
NEG_BIG = -1.0e30
G = (S + 127) // 128  # groups of 128 dst nodes per core
SP = G * 128          # padded shard size
DUMMY_SRC = V         # dummy gather-table row
DUMMY_LOC = S         # dummy local row (scatter target for pad nodes)

# module-level knobs (test harness pokes these)
PROFILE = False
DEBUG = False
LAST_EXEC_NS = None
LAST_RESULTS = None


# ---------------------------------------------------------------------------
# host-side graph preprocessing (integer work only)
# ---------------------------------------------------------------------------

def _host_prep(src, dst):
    src = np.asarray(src).astype(np.int64)
    dst = np.asarray(dst).astype(np.int64)
    order = np.argsort(dst, kind="stable")
    src_s = src[order].astype(np.int32)
    dst_s = dst[order].astype(np.int32)
    bounds = np.searchsorted(dst_s, np.arange(N_CORES + 1) * S)
    inv_glob = np.zeros(V, np.int32)  # node id -> pi-position row in t2_full

    cores = []
    for c in range(N_CORES):
        e_src = src_s[bounds[c]:bounds[c + 1]]
        ldst = dst_s[bounds[c]:bounds[c + 1]] - c * S
        deg = np.bincount(ldst, minlength=S)
        perm = np.argsort(-deg, kind="stable").astype(np.int32)  # local ids, deg desc
        starts = np.concatenate([[0], np.cumsum(deg)[:-1]]).astype(np.int64)
        perm_pad = np.concatenate(
            [perm, np.full(SP - S, DUMMY_LOC, np.int32)])
        deg_pad = np.concatenate([deg, [0]])       # deg[DUMMY_LOC] = 0
        starts_pad = np.concatenate([starts, [0]])
        gmax = np.array([
            max(int(deg_pad[perm_pad[g * 128]]), 1) for g in range(G)
        ])
        inv_glob[c * S + perm] = c * (SP + 1) + np.arange(S, dtype=np.int32)
        cores.append(dict(e_src=e_src, perm_pad=perm_pad, deg_pad=deg_pad,
                          starts_pad=starts_pad, gmax=gmax))

    L_g = np.max(np.stack([c["gmax"] for c in cores]), axis=0).astype(np.int64)
    offs = np.concatenate([[0], np.cumsum(L_g)]).astype(np.int64)
    tot_l = int(offs[-1])

    slots_all, scat_all = [], []
    for c in range(N_CORES):
        cc = cores[c]
        slots = np.full((128, tot_l), DUMMY_SRC, np.int32)
        for g in range(G):
            ids = cc["perm_pad"][g * 128:(g + 1) * 128]
            dg = cc["deg_pad"][ids]
            st = cc["starts_pad"][ids]
            L = int(L_g[g])
            ar = np.arange(L)
            mask = ar[None, :] < dg[:, None]
            pos = np.minimum(st[:, None] + ar[None, :], len(cc["e_src"]) - 1)
            vals = cc["e_src"][pos] if len(cc["e_src"]) else np.zeros_like(pos, np.int32)
            gslots = np.where(mask, vals, DUMMY_SRC).astype(np.int32)
            slots[:, offs[g]:offs[g + 1]] = gslots
        scat = cc["perm_pad"].reshape(G, 128).T.astype(np.int32).copy()  # [128, G]
        slots_all.append(slots)
        scat_all.append(scat)
    # layer-2 slots: remap src node id -> its pi-position row in t2_full
    DUMMY2 = SP
    slots2_all = [np.where(s == DUMMY_SRC, DUMMY2, inv_glob[np.minimum(s, V - 1)])
                  .astype(np.int32) for s in slots_all]
    return slots_all, slots2_all, scat_all, L_g, offs, tot_l


# ---------------------------------------------------------------------------
# device program
# ---------------------------------------------------------------------------

def _build_program(L_g, offs, tot_l, split=True):
    nc = bass.Bass("TRN2", target_bir_lowering=False, debug=False,
                   num_devices=N_CORES)

    inp = {}
    def dram_in(name, shape, dt=F32):
        inp[name] = nc.dram_tensor(name, list(shape), dt, kind="ExternalInput").ap()
        return inp[name]

    xT = dram_in("xT", [F_IN, SP])
    slots2_d = dram_in("slots2", [128, tot_l], I32)
    W1_d = dram_in("W1e", [F_IN, HD1 + 2 * H1])
    W2_d = dram_in("W2", [HD1, C2])
    b1_d = dram_in("b1", [128, HD1])
    al2_d = dram_in("al2", [128, C2])
    ar2_d = dram_in("ar2", [128, C2])
    b2_d = dram_in("b2", [128, C2])
    ident_d = dram_in("ident", [128, 128])
    drow1_d = dram_in("drow1", [1, ROW1])
    drow2_d = dram_in("drow2", [1, ROW2])

    out_shard = nc.dram_tensor("out_shard", [SP, C2], F32, kind="ExternalOutput").ap()
    dbg = {}
    if DEBUG:
        for nm, shape in [("dbg_t1", [SP, ROW1]), ("dbg_er1", [SP, H1]),
                          ("dbg_t2", [SP, ROW2]), ("dbg_h", [SP, HD1]),
                          ("dbg_t1f", [V + 1, ROW1]),
                          ("dbg_F1", [128, 4096]), ("dbg_erg", [128, H1]),
                          ("dbg_A", [128, 512]), ("dbg_EX", [128, 512]),
                          ("dbg_U", [128, HD1])]:
            dbg[nm] = nc.dram_tensor(nm, shape, F32, kind="ExternalOutput").ap()

    with tile.TileContext(nc) as tc:
        with (
            tc.tile_pool(name="dram", bufs=1, space="DRAM") as dram,
            tc.tile_pool(name="const", bufs=1) as constp,
            tc.tile_pool(name="work", bufs=2) as work,
            tc.tile_pool(name="gath", bufs=3) as gath,
            tc.tile_pool(name="psum", bufs=2, space="PSUM") as psum,
        ):
            # ---- persistent DRAM tables -----------------------------------
            t1_shard = dram.tile([SP + 1, ROW1], F32)
            t1_full = dram.tile([N_CORES * (SP + 1), ROW1], F32)
            er1_loc = dram.tile([SP, H1], F32)
            t2_shard = dram.tile([SP + 1, ROW2], F32)
            t2_full = dram.tile([N_CORES * (SP + 1), ROW2], F32)
            # collective outputs in shared DRAM (per compiler guidance)
            t1_full[:].tensor.mls.addr_space = "Shared"
            t2_full[:].tensor.mls.addr_space = "Shared"

            # ---- constants into SBUF --------------------------------------
            _cn = [0]
            def const_load(src_ap, shape, dt=F32):
                _cn[0] += 1
                t = constp.tile(shape, dt, tag=f"const{_cn[0]}")
                nc.sync.dma_start(out=t[:], in_=src_ap)
                return t

            W1a = const_load(W1_d[0:128, :], [128, HD1 + 2 * H1])
            W1b = const_load(W1_d[128:256, :], [128, HD1 + 2 * H1])
            W2sb = const_load(W2_d[:, :], [HD1, C2])
            b1 = const_load(b1_d[:, :], [128, HD1])
            al2 = const_load(al2_d[:, :], [128, C2])
            ar2 = const_load(ar2_d[:, :], [128, C2])
            b2 = const_load(b2_d[:, :], [128, C2])
            ident = const_load(ident_d[:, :], [128, 128])
            slots2_sb = const_load(slots2_d[:, :], [128, tot_l], I32)
            er2_sb = constp.tile([128, G], F32)

            # dummy rows of the gather tables
            nc.sync.dma_start(out=t1_shard[SP:SP + 1, :], in_=drow1_d[:, :])
            nc.sync.dma_start(out=t2_shard[SP:SP + 1, :], in_=drow2_d[:, :])

            # ---- node phase: feat1/el1/er1 for own shard ------------------
            NB = 4  # node tiles per xT load (DMA batching)
            for n in range(G):
                if n % NB == 0:
                    nw = min(NB, G - n) * 128
                    cs4 = slice(n * 128, n * 128 + nw)
                    xa = work.tile([128, NB * 128], F32, tag="xa")
                    xb = work.tile([128, NB * 128], F32, tag="xb")
                    nc.scalar.dma_start(out=xa[:, 0:nw], in_=xT[0:128, cs4])
                    nc.scalar.dma_start(out=xb[:, 0:nw], in_=xT[128:256, cs4])
                cs = slice(n * 128, (n + 1) * 128)
                k = (n % NB) * 128
                p1 = psum.tile([128, HD1 + 2 * H1], F32, tag="p1")
                nc.tensor.matmul(out=p1[:], lhsT=xa[:, k:k + 128], rhs=W1a[:],
                                 start=True, stop=False)
                nc.tensor.matmul(out=p1[:], lhsT=xb[:, k:k + 128], rhs=W1b[:],
                                 start=False, stop=True)
                if n % NB == 0:
                    S4 = work.tile([128, NB * ROW1], F32, tag="S4")
                    er4 = work.tile([128, NB * H1], F32, tag="er4")
                j = n % NB
                nc.scalar.copy(out=S4[:, j * ROW1:(j + 1) * ROW1],
                               in_=p1[:, 0:ROW1])
                nc.vector.tensor_copy(out=er4[:, j * H1:(j + 1) * H1],
                                      in_=p1[:, ROW1:ROW1 + H1])
                if n % NB == NB - 1 or n == G - 1:
                    m = n % NB + 1
                    a = (n - m + 1) * 128
                    nc.sync.dma_start(
                        out=t1_shard[a:a + m * 128, :]
                            .rearrange("(j p) r -> p j r", p=128),
                        in_=S4[:, 0:m * ROW1]
                            .rearrange("p (j r) -> p j r", r=ROW1))
                    nc.sync.dma_start(
                        out=er1_loc[a:a + m * 128, :]
                            .rearrange("(j p) r -> p j r", p=128),
                        in_=er4[:, 0:m * H1]
                            .rearrange("p (j r) -> p j r", r=H1))

            # ---- AllGather layer-1 gather table ---------------------------
            nc.gpsimd.collective_compute(
                "AllGather", OP.bypass,
                replica_groups=[list(range(N_CORES))],
                ins=[t1_shard[0:SP + 1, :].opt()],
                outs=[t1_full[:, :].opt()],
            )

            # ---- layer-1 edge phase (+ fused layer-2 projection) ----------
            for g in range(G):
                L = int(L_g[g])
                o0 = int(offs[g])
                F1 = gath.tile([128, L * ROW1], F32, tag="F1")
                for l in range(L):
                    nc.gpsimd.indirect_dma_start(
                        out=F1[:, l * ROW1:(l + 1) * ROW1], out_offset=None,
                        in_=t1_full[:, :],
                        in_offset=bass.IndirectOffsetOnAxis(
                            ap=slots2_sb[:, o0 + l:o0 + l + 1], axis=0))
                er_g = work.tile([128, H1], F32, tag="er_g")
                nc.sync.dma_start(
                    out=er_g[:], in_=er1_loc[g * 128:(g + 1) * 128, :])
                F1v = F1[:].rearrange("p (l r) -> p l r", r=ROW1)
                A = gath.tile([128, L * H1], F32, tag="A")
                nc.vector.tensor_add(
                    out=A[:].rearrange("p (l h) -> p l h", h=H1),
                    in0=F1v[:, :, HD1:ROW1],
                    in1=er_g[:].rearrange("p (o h) -> p o h", o=1)
                        .to_broadcast([128, L, H1]))
                AL = gath.tile([128, L * H1], F32, tag="AL")
                nc.vector.tensor_scalar_mul(out=AL[:], in0=A[:], scalar1=NEG_SLOPE)
                nc.vector.tensor_tensor(out=A[:], in0=A[:], in1=AL[:], op=OP.max)
                m2n = work.tile([128, 1], F32, tag="m2n")
                nc.vector.reduce_max(out=m2n[:], in_=A[:], axis=AX.X,
                                     negate=True)
                EX = gath.tile([128, L * H1], F32, tag="EX")
                nc.scalar.activation(out=EX[:], in_=A[:], func=AF.Exp,
                                     bias=m2n[:, 0:1])
                s_t = work.tile([128, H1], F32, tag="s_t")
                nc.vector.reduce_sum(
                    out=s_t[:],
                    in_=EX[:].rearrange("p (l h) -> p h l", h=H1), axis=AX.X)
                rinv = work.tile([128, H1], F32, tag="rinv")
                nc.vector.reciprocal(out=rinv[:], in_=s_t[:])
                P = gath.tile([128, L * HD1], F32, tag="P")
                nc.vector.tensor_mul(
                    out=P[:].rearrange("p (l h j) -> p l h j", h=H1, j=D1),
                    in0=F1v[:, :, 0:HD1].rearrange("p l (h j) -> p l h j", h=H1),
                    in1=EX[:].rearrange("p (l h) -> p l h", h=H1)
                        .rearrange("p l (h o) -> p l h o", o=1)
                        .to_broadcast([128, L, H1, D1]))
                U = work.tile([128, HD1], F32, tag="U")
                nc.vector.reduce_sum(
                    out=U[:],
                    in_=P[:].rearrange("p (l h j) -> p h j l", h=H1, j=D1),
                    axis=AX.X)
                Ht = work.tile([128, HD1], F32, tag="Ht")
                if DEBUG and g == 0:
                    nF = min(L * ROW1, 4096)
                    nc.sync.dma_start(out=dbg["dbg_F1"][:, 0:nF], in_=F1[:, 0:nF])
                    nc.sync.dma_start(out=dbg["dbg_erg"][:, :], in_=er_g[:])
                    nA = min(L * H1, 512)
                    nc.sync.dma_start(out=dbg["dbg_A"][:, 0:nA], in_=A[:, 0:nA])
                    nc.sync.dma_start(out=dbg["dbg_EX"][:, 0:nA], in_=EX[:, 0:nA])
                    nc.sync.dma_start(out=dbg["dbg_U"][:, :], in_=U[:])
                nc.vector.tensor_mul(
                    out=Ht[:],
                    in0=U[:],
                    in1=rinv[:].rearrange("p (h o) -> p h o", o=1)
                        .to_broadcast([128, H1, D1]))
                nc.vector.tensor_add(out=Ht[:], in0=Ht[:], in1=b1[:])
                nc.scalar.activation(out=Ht[:], in_=Ht[:], func=AF.Relu)
                if DEBUG:
                    nc.sync.dma_start(
                        out=dbg["dbg_h"][g * 128:(g + 1) * 128, :], in_=Ht[:])
                # layer-2 projection for these 128 nodes
                pT = psum.tile([HD1, 128], F32, tag="pT")
                nc.tensor.transpose(out=pT[:], in_=Ht[:], identity=ident[:])
                hT = work.tile([HD1, 128], F32, tag="hT")
                nc.scalar.copy(out=hT[:], in_=pT[:])
                p2 = psum.tile([128, C2], F32, tag="p2")
                nc.tensor.matmul(out=p2[:], lhsT=hT[:], rhs=W2sb[:],
                                 start=True, stop=True)
                S2 = work.tile([128, ROW2], F32, tag="S2")
                nc.scalar.copy(out=S2[:, 0:C2], in_=p2[:])
                q = work.tile([128, C2], F32, tag="q")
                nc.vector.tensor_mul(out=q[:], in0=S2[:, 0:C2], in1=al2[:])
                nc.vector.reduce_sum(out=S2[:, C2:C2 + 1], in_=q[:], axis=AX.X)
                nc.vector.memset(S2[:, C2 + 1:ROW2], 0.0)
                q2 = work.tile([128, C2], F32, tag="q2")
                nc.vector.tensor_mul(out=q2[:], in0=S2[:, 0:C2], in1=ar2[:])
                nc.vector.reduce_sum(out=er2_sb[:, g:g + 1], in_=q2[:], axis=AX.X)
                nc.sync.dma_start(
                    out=t2_shard[g * 128:(g + 1) * 128, :], in_=S2[:])

            # ---- AllGather layer-2 gather table ---------------------------
            nc.gpsimd.collective_compute(
                "AllGather", OP.bypass,
                replica_groups=[list(range(N_CORES))],
                ins=[t2_shard[0:SP + 1, :].opt()],
                outs=[t2_full[:, :].opt()],
            )

            # ---- layer-2 edge phase --------------------------------------
            for g in range(G):
                L = int(L_g[g])
                o0 = int(offs[g])
                F2 = gath.tile([128, L * ROW2], F32, tag="F2")
                for l in range(L):
                    nc.gpsimd.indirect_dma_start(
                        out=F2[:, l * ROW2:(l + 1) * ROW2], out_offset=None,
                        in_=t2_full[:, :],
                        in_offset=bass.IndirectOffsetOnAxis(
                            ap=slots2_sb[:, o0 + l:o0 + l + 1], axis=0))
                F2v = F2[:].rearrange("p (l r) -> p l r", r=ROW2)
                A2 = gath.tile([128, L], F32, tag="A2")
                nc.vector.tensor_add(
                    out=A2[:].rearrange("p (l o) -> p l o", o=1),
                    in0=F2v[:, :, C2:C2 + 1],
                    in1=er2_sb[:, g:g + 1].rearrange("p (o h) -> p o h", o=1)
                        .to_broadcast([128, L, 1]))
                AL2 = gath.tile([128, L], F32, tag="AL2")
                nc.vector.tensor_scalar_mul(out=AL2[:], in0=A2[:], scalar1=NEG_SLOPE)
                nc.vector.tensor_tensor(out=A2[:], in0=A2[:], in1=AL2[:], op=OP.max)
                m2n2 = work.tile([128, 1], F32, tag="m2n2")
                nc.vector.reduce_max(out=m2n2[:], in_=A2[:], axis=AX.X,
                                     negate=True)
                EX2 = gath.tile([128, L], F32, tag="EX2")
                nc.scalar.activation(out=EX2[:], in_=A2[:], func=AF.Exp,
                                     bias=m2n2[:, 0:1])
                s2_t = work.tile([128, 1], F32, tag="s2_t")
                nc.vector.reduce_sum(out=s2_t[:], in_=EX2[:], axis=AX.X)
                rinv2 = work.tile([128, 1], F32, tag="rinv2")
                nc.vector.reciprocal(out=rinv2[:], in_=s2_t[:])
                P2 = gath.tile([128, L * C2], F32, tag="P2")
                nc.vector.tensor_mul(
                    out=P2[:].rearrange("p (l k) -> p l k", k=C2),
                    in0=F2v[:, :, 0:C2],
                    in1=EX2[:].rearrange("p (l o) -> p l o", o=1)
                        .to_broadcast([128, L, C2]))
                U2 = work.tile([128, C2], F32, tag="U2")
                nc.vector.reduce_sum(
                    out=U2[:],
                    in_=P2[:].rearrange("p (l k) -> p k l", k=C2), axis=AX.X)
                O = work.tile([128, C2], F32, tag="O")
                nc.vector.tensor_mul(
                    out=O[:], in0=U2[:],
                    in1=rinv2[:, 0:1].to_broadcast([128, C2]))
                nc.vector.tensor_add(out=O[:], in0=O[:], in1=b2[:])
                nc.sync.dma_start(
                    out=out_shard[g * 128:(g + 1) * 128, :], in_=O[:])
            if DEBUG:
                nc.sync.dma_start(out=dbg["dbg_t1"][:, :], in_=t1_shard[:, :])
                nc.sync.dma_start(out=dbg["dbg_er1"][:, :], in_=er1_loc[:, :])
                nc.sync.dma_start(out=dbg["dbg_t2"][:, :], in_=t2_shard[:, :])
                nc.sync.dma_start(out=dbg["dbg_t1f"][:, :], in_=t1_full[:, :])

    if split:
        _split_waits(nc)
    return nc


# ---------------------------------------------------------------------------
# entry point
# ---------------------------------------------------------------------------

def kernel(x, W1, attn_l1, attn_r1, b1, W2, attn_l2, attn_r2, b2, src, dst):
    global LAST_EXEC_NS
    x = np.asarray(x, np.float32)
    slots_all, slots2_all, scat_all, L_g, offs, tot_l = _host_prep(src, dst)
    nc = _build_program(L_g, offs, tot_l)

    drow1 = np.zeros((1, ROW1), np.float32)
    drow1[0, HD1:] = NEG_BIG
    drow2 = np.zeros((1, ROW2), np.float32)
    drow2[0, C2] = NEG_BIG

    W1f = np.asarray(W1, np.float32)
    al1f = np.asarray(attn_l1, np.float32).reshape(H1, D1)
    ar1f = np.asarray(attn_r1, np.float32).reshape(H1, D1)
    Wl = (W1f.reshape(F_IN, H1, D1) * al1f[None]).sum(-1).astype(np.float32)
    Wr = (W1f.reshape(F_IN, H1, D1) * ar1f[None]).sum(-1).astype(np.float32)
    common = {
        "W1e": np.concatenate([W1f, Wl, Wr], axis=1),
        "W2": np.asarray(W2, np.float32),
        "b1": np.tile(np.asarray(b1, np.float32).reshape(1, HD1), (128, 1)),
        "al2": np.tile(np.asarray(attn_l2, np.float32).reshape(1, C2), (128, 1)),
        "ar2": np.tile(np.asarray(attn_r2, np.float32).reshape(1, C2), (128, 1)),
        "b2": np.tile(np.asarray(b2, np.float32).reshape(1, C2), (128, 1)),
        "ident": np.eye(128, dtype=np.float32),
        "drow1": drow1,
        "drow2": drow2,
    }
    in_maps = []
    for c in range(N_CORES):
        xs = np.zeros((F_IN, SP), np.float32)
        perm_pad = scat_all[c].T.reshape(-1)
        valid = perm_pad < S
        xs[:, valid] = x[c * S + perm_pad[valid]].T
        in_maps.append({"xT": np.ascontiguousarray(xs),
                        "slots2": slots2_all[c], **common})

    res = run_bass_kernel_spmd(nc, in_maps, core_ids=list(range(N_CORES)),
                               trace=PROFILE)
    global LAST_RESULTS
    LAST_RESULTS = res.results
    LAST_EXEC_NS = res.exec_time_ns
    # out_shard rows are in pi (degree-sorted) order; un-permute on host
    out = np.zeros((V, C2), np.float32)
    for c in range(N_CORES):
        perm_pad = scat_all[c].T.reshape(-1)       # [SP] local ids (S = pad)
        valid = perm_pad < S
        out[c * S + perm_pad[valid]] = res.results[c]["out_shard"][valid]
    return out

